# revision 7
# baseline (speedup 1.0000x reference)
"""Additive (Bahdanau) attention on 8 TRN2 NeuronCores — self-contained Bass kernel.

Math: score(q,k) = w2 . tanh(hq[q] + hk[k] + b1) + b2;  out = softmax_k(score) @ V.

tanh(s) ~= sum_m c_m sin(w_m s) with a DOUBLING basis w_m = {1,2,4,8}*w0
(weighted-LSQ fit; e2e rel-err ~4.6e-3 in full bf16 simulation).  Angle
addition sin(w(a+b)) = sin(wa)cos(wb)+cos(wa)sin(wb) turns the [B,Q,K,D]
tanh+reduce into TensorE matmuls contracting over (2M x D).

Per side (F=queries, G=keys; b1 folds into the G-side Sin bias), features at
level scales [alpha]:
  m0: s1=sin(w0 h) [1],   c1=sin(w0 h + pi/2) [1]     (ScalarE, from PSUM)
  m1: S2=s1*c1 [1/2],     C2=0.5-s1^2 [1/2]           (DVE bf16)
  m2: S4=S2*C2 [1/8],     C4=0.125-S2^2 [1/8]
  m3: S8=S4*C4 [1/128],   C8=1/128-S4^2 [1/128]
No range reduction, no |h| pass, all Sin args within [-pi,pi] (CoreSim-safe).
Scale products absorb into the F-side per-partition multiplier
c_m*w2_d/alpha_m^2.

Softmax: b2 drops (shift invariance); the denominator comes FREE from attn@V
by appending a ones-column to V ([P,257] matmul), reciprocal on VectorE,
folded into a per-q output scale.

Structure: inputs split over 4 DMA queues (sync/tensor/vector/gpsimd); a
dummy 1-col Sin hoists the trig ACT-table load to t~0; Sin reads h straight
from PSUM (no h copy); the exp table load overlaps mid-kernel compute.

Sharding: data-parallel over batch, B=16 -> 2 per core, no collectives.
"""

import math
from contextlib import ExitStack

import numpy as np
import ml_dtypes

import concourse.bass as bass
import concourse.mybir as mybir
import concourse.tile as tile
from concourse import bacc
from concourse.bass_utils import run_bass_kernel_spmd

F32 = mybir.dt.float32
BF16 = mybir.dt.bfloat16
AF = mybir.ActivationFunctionType
ALU = mybir.AluOpType

NCORES = 8
B, NQ, NK, D = 16, 256, 256, 256
BL = B // NCORES          # local batches per core = 2
P = 128
DC = D // P               # d-chunks = 2
EC = D // P               # e-chunks (contraction for hq/hk matmuls) = 2
QT = NQ // P              # q-tiles = 2
KT = NK // P              # k-tiles = 2
M_SINES = 4
W = BL * NQ               # 512: free width per dt slice
WF = DC * W               # 1024: per-side width (F half [0,WF), G half [WF,2WF))
DV1 = D + 1               # values + ones column

# {1,2,4,8}*W0 weighted-LSQ fit of tanh (Gaussian(~1.0)+5e-4 weight)
W0 = 0.378
COEF = (1.186435, 0.13547, 0.228208, 0.032448)
ALPHA2 = (1.0, 0.25, 1.0 / 64, 1.0 / 16384)   # alpha_sin*alpha_cos per m

# tbl columns
TB_WB1 = 0                 # [dt]      w0*b1
TB_HPI = DC                # [1]       pi/2
TB_WB1H = DC + 1           # [dt]      w0*b1 + pi/2
TB_W2C = 2 * DC + 1        # [m*DC+dt] c_m*w2/alpha2_m
TB_N = TB_W2C + M_SINES * DC


def build_kernel() -> bacc.Bacc:
    nc = bacc.Bacc("TRN2", target_bir_lowering=False, debug=False)

    q_d = nc.dram_tensor("queries", [BL, NQ, D], BF16, kind="ExternalInput").ap()
    k_d = nc.dram_tensor("keys", [BL, NK, D], BF16, kind="ExternalInput").ap()
    v_d = nc.dram_tensor("values", [BL, NK, D], BF16, kind="ExternalInput").ap()
    wqk_d = nc.dram_tensor("wqk", [P, 2 * EC * D], BF16, kind="ExternalInput").ap()
    tbl_d = nc.dram_tensor("tbl", [P, TB_N], F32, kind="ExternalInput").ap()
    id_d = nc.dram_tensor("ident", [P, P], BF16, kind="ExternalInput").ap()
    out_d = nc.dram_tensor("out", [BL, NQ, D], F32, kind="ExternalOutput").ap()

    with tile.TileContext(nc) as tc, ExitStack() as ctx:
        cpool = ctx.enter_context(tc.tile_pool(name="consts", bufs=1))
        dpool = ctx.enter_context(tc.tile_pool(name="data", bufs=1))

        # dummy 1-col Sin: hoists the trig ACT-table load to program start
        dummy = cpool.tile([P, 2], F32)
        nc.vector.memset(dummy[:, 0:1], 0.0)
        nc.scalar.activation(dummy[:, 1:2], dummy[:, 0:1], AF.Sin)

        ident = cpool.tile([P, P], BF16)
        wqk = cpool.tile([P, 2 * EC * D], BF16)
        tbl = cpool.tile([P, TB_N], F32)
        qnb = dpool.tile([P, BL * QT * D], BF16)
        knb = dpool.tile([P, BL * KT * D], BF16)
        vb = dpool.tile([P, BL * KT * DV1], BF16)

        # ---- input DMAs: 3 queues (sync/scalar/gpsimd) in parallel ----
        nc.sync.dma_start(ident[:], id_d[:])
        nc.sync.dma_start(
            qnb[:, 0:QT * D].rearrange("p (t e) -> p t e", t=QT),
            q_d[0].rearrange("(t p) e -> p t e", p=P))
        nc.scalar.dma_start(
            qnb[:, QT * D:2 * QT * D].rearrange("p (t e) -> p t e", t=QT),
            q_d[1].rearrange("(t p) e -> p t e", p=P))
        nc.scalar.dma_start(wqk[:], wqk_d[:])
        nc.gpsimd.dma_start(tbl[:], tbl_d[:])
        nc.gpsimd.dma_start(
            knb[:, 0:KT * D].rearrange("p (t e) -> p t e", t=KT),
            k_d[0].rearrange("(t p) e -> p t e", p=P))
        nc.sync.dma_start(
            knb[:, KT * D:2 * KT * D].rearrange("p (t e) -> p t e", t=KT),
            k_d[1].rearrange("(t p) e -> p t e", p=P))
        nc.gpsimd.dma_start(
            vb[:].rearrange("p (b t e) -> p b t e", b=BL, t=KT)[:, :, :, 0:D],
            v_d.rearrange("b (t p) e -> p b t e", p=P))
        nc.gpsimd.memset(
            vb[:].rearrange("p (b t e) -> p b t e", b=BL, t=KT)[:, :, :, D:DV1], 1.0)

        halfpi = tbl[:, TB_HPI:TB_HPI + 1]

        # transposed inputs (bf16): col = (ec*BL + b)*256 + q
        qTt = dpool.tile([P, EC * BL * NQ], BF16)
        kTt = dpool.tile([P, EC * BL * NK], BF16)

        # trig feature tiles: F half [0,WF), G half [WF,2WF);
        # within a half: col = dt*W + b*NQ + q
        s1 = dpool.tile([P, 2 * WF], BF16)
        c1 = dpool.tile([P, 2 * WF], BF16)
        S2 = dpool.tile([P, 2 * WF], BF16)
        C2 = dpool.tile([P, 2 * WF], BF16)
        S4 = dpool.tile([P, 2 * WF], BF16)
        C4 = dpool.tile([P, 2 * WF], BF16)
        S8 = dpool.tile([P, 2 * WF], BF16)
        C8 = dpool.tile([P, 2 * WF], BF16)
        ppool = ctx.enter_context(tc.tile_pool(name="prods", bufs=2))

        wpool = ctx.enter_context(tc.tile_pool(name="wpsum", bufs=2, space="PSUM"))
        sfpool = ctx.enter_context(tc.tile_pool(name="scaledF", bufs=2))

        with tc.tile_pool(name="tpsum", bufs=2, space="PSUM") as tpool:
            # transposes: q side then k side (PE); copies: q on ScalarE, k on DVE
            tps = {}
            for (side, natb, nt) in ((0, qnb, QT), (1, knb, KT)):
                for j in range(EC):
                    tp = tpool.tile([P, BL * QT * P], BF16, name=f"tp{side}{j}", tag="tp")
                    tps[(side, j)] = tp
                    for b in range(BL):
                        for i in range(nt):
                            nc.tensor.transpose(
                                tp[:, (b * nt + i) * P:(b * nt + i + 1) * P],
                                natb[:, (b * nt + i) * D + j * P:(b * nt + i) * D + (j + 1) * P],
                                ident)
                    if side == 0:
                        nc.scalar.activation(
                            qTt[:, j * BL * NQ:(j + 1) * BL * NQ], tp[:], AF.Copy)
                    else:
                        nc.vector.tensor_copy(
                            kTt[:, j * BL * NQ:(j + 1) * BL * NQ], tp[:])

        with tc.tile_pool(name="hpsum", bufs=2, space="PSUM") as hpool:
            # hq then hk matmuls into PSUM; Sin reads PSUM directly
            h_f = hpool.tile([P, 2 * W], F32, name="h_f", tag="h")
            h_g = hpool.tile([P, 2 * W], F32, name="h_g", tag="h")
            for (h_ps, dst, woff) in ((h_f, qTt, 0), (h_g, kTt, EC * D)):
                for dt in range(DC):
                    for b in range(BL):
                        for ec in range(EC):
                            nc.tensor.matmul(
                                h_ps[:, dt * W + b * NQ:dt * W + (b + 1) * NQ],
                                wqk[:, woff + ec * D + dt * P:woff + ec * D + (dt + 1) * P],
                                dst[:, (ec * BL + b) * NQ:(ec * BL + b + 1) * NQ],
                                start=(ec == 0), stop=(ec == EC - 1))

            # base trig straight from PSUM (F: one op per fn; G: per-dt, b1 in bias)
            nc.scalar.activation(s1[:, 0:WF], h_f[:], AF.Sin, bias=0.0, scale=W0)
            nc.scalar.activation(c1[:, 0:WF], h_f[:], AF.Sin, bias=halfpi, scale=W0)
            for dt in range(DC):
                nc.scalar.activation(
                    s1[:, WF + dt * W:WF + (dt + 1) * W], h_g[:, dt * W:(dt + 1) * W],
                    AF.Sin, bias=tbl[:, TB_WB1 + dt:TB_WB1 + dt + 1], scale=W0)
                nc.scalar.activation(
                    c1[:, WF + dt * W:WF + (dt + 1) * W], h_g[:, dt * W:(dt + 1) * W],
                    AF.Sin, bias=tbl[:, TB_WB1H + dt:TB_WB1H + dt + 1], scale=W0)

        # doubling chain on DVE, F side first (its SINs finish first), with
        # F-side scaled copies (sF) interleaved right after each level
        logits_ps = [wpool.tile([P, BL * NQ], F32, name=f"lg_{kt}", tag="lg")
                     for kt in range(KT)]
        expT = dpool.tile([P, KT * BL * NQ], BF16)
        GFEAT = ((s1, c1), (S2, C2), (S4, C4), (S8, C8))
        sFs = [sfpool.tile([P, 2 * WF], BF16, name=f"sF{m}", tag=f"sF{m % 2}")
               for m in range(M_SINES)]

        if True:
            def chain_side(lo):
                sl = slice(lo, lo + WF)
                pa = ppool.tile([P, WF], BF16, name="pa", tag="pp")
                nc.vector.tensor_tensor(pa[:], s1[:, sl], s1[:, sl], op=ALU.mult)
                nc.vector.tensor_scalar(C2[:, sl], pa[:], -1.0, 0.5,
                                        op0=ALU.mult, op1=ALU.add)
                nc.vector.tensor_tensor(S2[:, sl], s1[:, sl], c1[:, sl], op=ALU.mult)
                pb = ppool.tile([P, WF], BF16, name="pb", tag="pp")
                nc.vector.tensor_tensor(pb[:], S2[:, sl], S2[:, sl], op=ALU.mult)
                nc.vector.tensor_scalar(C4[:, sl], pb[:], -1.0, 0.125,
                                        op0=ALU.mult, op1=ALU.add)
                nc.vector.tensor_tensor(S4[:, sl], S2[:, sl], C2[:, sl], op=ALU.mult)
                pc = ppool.tile([P, WF], BF16, name="pc", tag="pp")
                nc.vector.tensor_tensor(pc[:], S4[:, sl], S4[:, sl], op=ALU.mult)
                nc.vector.tensor_scalar(C8[:, sl], pc[:], -1.0, 1.0 / 128,
                                        op0=ALU.mult, op1=ALU.add)
                nc.vector.tensor_tensor(S8[:, sl], S4[:, sl], C4[:, sl], op=ALU.mult)

            def scale_m(m):
                gsin, gcos = GFEAT[m]
                for dt in range(DC):
                    col = TB_W2C + m * DC + dt
                    nc.vector.tensor_scalar_mul(
                        sFs[m][:, dt * W:(dt + 1) * W],
                        gsin[:, dt * W:(dt + 1) * W], tbl[:, col:col + 1])
                    nc.vector.tensor_scalar_mul(
                        sFs[m][:, WF + dt * W:WF + (dt + 1) * W],
                        gcos[:, dt * W:(dt + 1) * W], tbl[:, col:col + 1])

            def logits_m(m, last):
                gsin, gcos = GFEAT[m]
                if not last:
                    for (pi_, gt) in ((0, gcos), (1, gsin)):
                        for dt in range(DC):
                            for b in range(BL):
                                for kt in range(KT):
                                    nc.tensor.matmul(
                                        logits_ps[kt][:, b * NQ:(b + 1) * NQ],
                                        gt[:, WF + dt * W + b * NQ + kt * P:WF + dt * W + b * NQ + (kt + 1) * P],
                                        sFs[m][:, pi_ * WF + dt * W + b * NQ:pi_ * WF + dt * W + (b + 1) * NQ],
                                        start=(m == 0 and pi_ == 0 and dt == 0),
                                        stop=False)
                else:
                    # close kt=0's accumulation first so its EXP starts early
                    for kt in range(KT):
                        for (pi_, gt) in ((0, gcos), (1, gsin)):
                            for dt in range(DC):
                                for b in range(BL):
                                    nc.tensor.matmul(
                                        logits_ps[kt][:, b * NQ:(b + 1) * NQ],
                                        gt[:, WF + dt * W + b * NQ + kt * P:WF + dt * W + b * NQ + (kt + 1) * P],
                                        sFs[m][:, pi_ * WF + dt * W + b * NQ:pi_ * WF + dt * W + (b + 1) * NQ],
                                        start=False,
                                        stop=(pi_ == 1 and dt == DC - 1 and b == BL - 1))
                        nc.scalar.activation(
                            expT[:, kt * BL * NQ:(kt + 1) * BL * NQ],
                            logits_ps[kt][:], AF.Exp)

            scale_m(0)            # needs only s1F/c1F
            chain_side(0)         # F chain
            scale_m(1)
            logits_m(0, False)
            chain_side(WF)        # G chain
            scale_m(2)
            scale_m(3)
            logits_m(1, False)
            logits_m(2, False)
            logits_m(3, True)

        # ---- attn @ [V|1]: denominator rides in column 256 ----
        out_sb = dpool.tile([P, BL * QT * D], F32)
        rcol = cpool.tile([P, BL * QT], F32)
        for qt in range(QT):
            for b in range(BL):
                av = wpool.tile([P, DV1], F32, name=f"av_{qt}_{b}", tag="av")
                for kt in range(KT):
                    nc.tensor.matmul(
                        av[:],
                        expT[:, (kt * BL + b) * NQ + qt * P:(kt * BL + b) * NQ + (qt + 1) * P],
                        vb[:, (b * KT + kt) * DV1:(b * KT + kt + 1) * DV1],
                        start=(kt == 0), stop=(kt == KT - 1))
                rc = rcol[:, b * QT + qt:b * QT + qt + 1]
                nc.vector.reciprocal(rc, av[:, D:DV1])
                osl = out_sb[:, (b * QT + qt) * D:(b * QT + qt + 1) * D]
                if (b * QT + qt) % 2 == 0:
                    nc.scalar.activation(osl, av[:, 0:D], AF.Copy, bias=0.0, scale=rc)
                    nc.sync.dma_start(out_d[b, qt * P:(qt + 1) * P, :], osl)
                else:
                    nc.vector.tensor_scalar_mul(osl, av[:, 0:D], rc)
                    nc.gpsimd.dma_start(out_d[b, qt * P:(qt + 1) * P, :], osl)

    nc.compile()
    return nc


def _host_tables(b1: np.ndarray, w2: np.ndarray):
    tbl = np.zeros((P, TB_N), np.float32)
    tbl[:, TB_HPI] = math.pi / 2.0
    for dt in range(DC):
        tbl[:, TB_WB1 + dt] = W0 * b1[dt * P:(dt + 1) * P]
        tbl[:, TB_WB1H + dt] = W0 * b1[dt * P:(dt + 1) * P] + math.pi / 2.0
        for mi in range(M_SINES):
            tbl[:, TB_W2C + mi * DC + dt] = COEF[mi] * w2[dt * P:(dt + 1) * P] / ALPHA2[mi]
    return tbl


_NC_CACHE = {}


def _get_nc():
    if "nc" not in _NC_CACHE:
        _NC_CACHE["nc"] = build_kernel()
    return _NC_CACHE["nc"]


def _make_in_maps(inputs):
    keys = np.ascontiguousarray(np.asarray(inputs["keys"], np.float32).astype(ml_dtypes.bfloat16))
    queries = np.ascontiguousarray(np.asarray(inputs["queries"], np.float32).astype(ml_dtypes.bfloat16))
    values = np.ascontiguousarray(np.asarray(inputs["values"], np.float32).astype(ml_dtypes.bfloat16))
    Wk = np.asarray(inputs["Wk"], np.float32)
    Wq = np.asarray(inputs["Wq"], np.float32)
    b1 = np.asarray(inputs["b1"], np.float64)
    w2 = np.asarray(inputs["w2"], np.float64)

    wqk = np.concatenate(
        [Wq.reshape(EC, P, D).transpose(1, 0, 2).reshape(P, EC * D),
         Wk.reshape(EC, P, D).transpose(1, 0, 2).reshape(P, EC * D)],
        axis=1).astype(ml_dtypes.bfloat16)
    wqk = np.ascontiguousarray(wqk)
    tbl = _host_tables(b1, w2)
    ident = np.ascontiguousarray(np.eye(P, dtype=np.float32).astype(ml_dtypes.bfloat16))

    in_maps = []
    for c in range(NCORES):
        sl = slice(c * BL, (c + 1) * BL)
        in_maps.append({
            "queries": queries[sl], "keys": keys[sl], "values": values[sl],
            "wqk": wqk, "tbl": tbl, "ident": ident,
        })
    return in_maps


def _run(inputs, trace=False, trace_kwargs=None):
    nc = _get_nc()
    in_maps = _make_in_maps(inputs)
    kwargs = {}
    if trace:
        kwargs = dict(trace=True, trace_cores=[0], trace_kwargs=trace_kwargs or {})
    res = run_bass_kernel_spmd(nc, in_maps, core_ids=list(range(NCORES)), **kwargs)
    out = np.concatenate([res.results[c]["out"] for c in range(NCORES)], axis=0)
    return out, res


def kernel(**inputs) -> np.ndarray:
    out, _ = _run(inputs, trace=False)
    return out
